# revision 35
# baseline (speedup 1.0000x reference)
"""Bidirectional attention block (RMSNorm -> QKV+RoPE -> SDPA -> out-proj -> residual)
on 8 Trainium2 NeuronCores.

Sharding: tensor-parallel over heads (2 heads/core) through attention, then two
per-batch on-device AllToAlls switch to token-parallel (2x256 tokens/core) for
the output projection + residual. Host only slices/concatenates numpy arrays.

Key structure (v3):
- x loaded ONCE (feature-major xT), one batched DMA per 512-token group.
- Token sum-of-squares for RMSNorm computed on the PE via X^T.X diagonal
  (DVE multiply-by-identity + row reduce extracts the diag).
- RoPE rotate-half via a PE matmul with a constant +-1 permutation matrix.
- v transposed to token-major via PE transposes (not DMA).
- Attention AV uses the transposed orientation: out[q, hd] accumulates in PSUM
  with an extra ones-column producing the softmax denominator; normalization is
  a per-partition broadcast multiply (no DMA round trips).
- The kt loop is software-pipelined (scores for kt+1 issue before AV of kt) and
  batch-1 prep / batch-0 out-projection are sliced into small "filler" units
  interleaved between attention iterations to keep the PE busy during exp.

Shapes hardcoded for B=2, T=2048, D_MODEL=1024, N_HEADS=16, HEAD_DIM=64.
"""

import numpy as np
import ml_dtypes

import concourse.bass as bass
import concourse.tile as tile
from concourse import bacc, mybir
from concourse.bass_utils import run_bass_kernel_spmd
from concourse.masks import make_identity

B, T, D = 2, 2048, 1024
H, HD = 16, 64
BT = B * T                      # 4096 tokens
N_CORES = 8
HPC = H // N_CORES              # 2 heads per core
JC = 3 * HPC * HD               # 384 qkv features per core
TD = BT // N_CORES // B         # 256 tokens per core per batch (stage D)
RMS_EPS = 1e-5
ROPE_BASE = 10000.0

BF = mybir.dt.bfloat16
F32 = mybir.dt.float32

QG = 512                        # queries per attention group
NQG = T // QG                   # 4 q-groups per batch
NKT = T // 128                  # 16 k-tiles per batch
NTT = BT // 128                 # 32 token tiles globally


def build_kernel(nc, with_collective=True):
    xT_ap = nc.dram_tensor("xT", [8, 128, BT], BF, kind="ExternalInput").ap()
    wq_ap = nc.dram_tensor("wq", [8, 128, JC], BF, kind="ExternalInput").ap()
    wo_ap = nc.dram_tensor("wo", [8, 128, D], BF, kind="ExternalInput").ap()
    tab_ap = nc.dram_tensor("tables", [128, 2 * T + 128], BF,
                            kind="ExternalInput").ap()
    xres_ap = nc.dram_tensor("xres", [2 * TD, D], BF, kind="ExternalInput").ap()
    y_ap = nc.dram_tensor("y", [2 * TD, D], BF, kind="ExternalOutput").ap()

    rrow_d = nc.dram_tensor("rms_row", [BT], BF).ap()

    with tile.TileContext(nc) as tc:
        _body(nc, tc, dict(
            xT=xT_ap, wq=wq_ap, wo=wo_ap, tab=tab_ap,
            xres=xres_ap, y=y_ap, rrow_d=rrow_d,
        ), with_collective)
    return nc


def _body(nc, tc, io, with_collective):
    from contextlib import ExitStack
    ctx = ExitStack()
    with ctx:
        singles = ctx.enter_context(tc.tile_pool(name="singles", bufs=1))
        xsl_pool = ctx.enter_context(tc.tile_pool(name="xsl", bufs=4))
        rope_tmp = ctx.enter_context(tc.tile_pool(name="rope_tmp", bufs=2))
        exp_pool = ctx.enter_context(tc.tile_pool(name="exp", bufs=4))
        epi_pool = ctx.enter_context(tc.tile_pool(name="epi", bufs=2))
        out_sb = ctx.enter_context(tc.tile_pool(name="out_sb", bufs=2))
        dram = ctx.enter_context(tc.tile_pool(name="dram", bufs=1, space="DRAM"))
        mm_ps = ctx.enter_context(tc.tile_pool(name="mm_ps", bufs=2, space="PSUM"))
        st_ps = ctx.enter_context(tc.tile_pool(name="st_ps", bufs=2, space="PSUM"))
        av_ps = ctx.enter_context(tc.tile_pool(name="av_ps", bufs=1, space="PSUM"))

        # ---- constants (batched loads; xsl0 issued first in the schedule) ----
        ident = singles.tile([128, 128], BF)
        make_identity(nc, ident)
        ident32 = singles.tile([128, 128], F32)
        make_identity(nc, ident32)
        tab_sb = singles.tile([128, 2 * T + 128], BF)
        wq_sb = singles.tile([128, 8 * JC], BF)

        def load_consts():
            nc.sync.dma_start(
                wq_sb.rearrange("p (c j) -> p c j", c=8),
                io["wq"].rearrange("c p j -> p c j"),
            )
            nc.sync.dma_start(tab_sb[:], io["tab"][:])

        cos_sb = tab_sb[:, 0:T]
        sin_sb = tab_sb[:, T : 2 * T]
        rotT = tab_sb[:, 2 * T : 2 * T + 128]
        eps_t = singles.tile([128, 1], F32)
        nc.vector.memset(eps_t[:], RMS_EPS)
        sumsq = singles.tile([128, NTT], F32)
        rms_tok = singles.tile([128, NTT], F32)
        rms_bc = [None, None]

        # ---- stage B: qkv projection + RoPE + sumsq via X^T.X diag ----
        blocks = []  # q, k, v blocks [128, BT] bf16
        for jt in range(3):
            blocks.append(singles.tile([128, BT], BF, tag=f"blk{jt}", name=f"blk{jt}"))

        def xsl_load(tg, staggered=False):
            g = slice(tg * 512, (tg + 1) * 512)
            t = xsl_pool.tile([128, 8 * 512], BF, tag="xsl", name="xsl")
            if staggered:
                for ch in range(8):
                    nc.sync.dma_start(t[:, ch * 512 : (ch + 1) * 512],
                                      io["xT"][ch, :, g])
            else:
                nc.sync.dma_start(
                    t.rearrange("p (c x) -> p c x", c=8),
                    io["xT"][:, :, g].rearrange("c p x -> p c x"),
                )
            return t

        def tg_xtx_units(tg, xsl):
            """sumsq for this tg's 4 token tiles: diag of X^T X."""
            for s in range(4):
                gt = tg * 4 + s
                ss = slice(s * 128, (s + 1) * 128)
                xtx = mm_ps.tile([128, 128], F32, tag="mmps", name="xtx")
                for ch in range(8):
                    nc.tensor.matmul(
                        xtx[:], lhsT=xsl[:, ch * 512 : (ch + 1) * 512][:, ss],
                        rhs=xsl[:, ch * 512 : (ch + 1) * 512][:, ss],
                        start=(ch == 0), stop=(ch == 7),
                    )
                junk = epi_pool.tile([128, 128], F32, tag="junk", name="junk")
                nc.vector.tensor_mul(junk[:], xtx[:], ident32[:])
                nc.vector.tensor_reduce(
                    sumsq[:, gt : gt + 1], junk[:],
                    axis=mybir.AxisListType.X, op=mybir.AluOpType.add,
                )
                yield

        def tg_qkv_units(tg, xsl):
            """qkv projection then RoPE for one 512-token group."""
            b = tg // (T // 512)
            tl = tg * 512 - b * T          # batch-local col offset
            g = slice(tg * 512, (tg + 1) * 512)
            cs = slice(tl, tl + 512)
            for jt in range(3):
                ps = mm_ps.tile([128, 512], F32, tag="mmps", name="qkvps")
                for ch in range(8):
                    nc.tensor.matmul(
                        ps[:],
                        lhsT=wq_sb[:, ch * JC + jt * 128 : ch * JC + (jt + 1) * 128],
                        rhs=xsl[:, ch * 512 : (ch + 1) * 512],
                        start=(ch == 0), stop=(ch == 7),
                    )
                nc.vector.tensor_copy(blocks[jt][:, g], ps[:])
                yield
            # RoPE on q, k slices in place
            for jt in (0, 1):
                blk = blocks[jt]
                rot_ps = mm_ps.tile([128, 512], F32, tag="mmps", name="rotps")
                nc.tensor.matmul(rot_ps[:], lhsT=rotT, rhs=blk[:, g],
                                 start=True, stop=True)
                m1 = rope_tmp.tile([128, 512], BF, tag="m1", name="m1")
                nc.gpsimd.tensor_mul(m1[:], blk[:, g], cos_sb[:, cs])
                m2 = rope_tmp.tile([128, 512], BF, tag="m2", name="m2")
                nc.vector.tensor_mul(m2[:], rot_ps[:], sin_sb[:, cs])
                nc.gpsimd.tensor_add(blk[:, g], m1[:], m2[:])
                yield

        def build_rms(b):
            bs = slice(b * (NTT // 2), (b + 1) * (NTT // 2))
            sqv = epi_pool.tile([128, NTT // 2], F32, tag="sqv", name="sqv")
            nc.scalar.activation(
                sqv[:], sumsq[:, bs], mybir.ActivationFunctionType.Sqrt,
                bias=eps_t[:], scale=1.0 / D,
            )
            nc.vector.reciprocal(rms_tok[:, bs], sqv[:])
            rtr_ps = mm_ps.tile([NTT // 2, 128], F32, tag="mmps", name="rtr")
            nc.tensor.transpose(rtr_ps[:], rms_tok[:, bs], ident32[:])
            rms_tr = epi_pool.tile([NTT // 2, 128], BF, tag="rmstr", name="rmstr")
            nc.vector.tensor_copy(rms_tr[:], rtr_ps[:])
            nc.sync.dma_start(
                io["rrow_d"].rearrange("(x k p) -> x k p", x=B, p=128)[b],
                rms_tr[:],
            )
            rbc = singles.tile([128, T], BF, tag=f"rmsbc{b}", name=f"rmsbc{b}")
            rrow_bcast = bass.AP(
                tensor=io["rrow_d"].tensor, offset=io["rrow_d"].offset + b * T,
                ap=[[0, 128], [1, T]],
            )
            nc.gpsimd.dma_start(rbc[:], rrow_bcast)
            rms_bc[b] = rbc

        def q_rms(tg):
            b = tg // (T // 512)
            tl = tg * 512 - b * T
            g = slice(tg * 512, (tg + 1) * 512)
            nc.vector.tensor_mul(blocks[0][:, g], blocks[0][:, g],
                                 rms_bc[b][:, tl : tl + 512])

        # ---- v transpose to token-major [128 tok, 2*(64 v + ones)] ----
        v_t = {}

        def build_vt_units(b):
            for kt in range(NKT):
                vps = mm_ps.tile([128, 128], BF, tag="mmps", name="vps")
                nc.tensor.transpose(
                    vps[:],
                    blocks[2][:, b * T + kt * 128 : b * T + (kt + 1) * 128],
                    ident[:],
                )
                vt = singles.tile([128, 130], BF, tag=f"vt{b}_{kt}",
                                  name=f"vt{b}_{kt}")
                for h in range(2):
                    nc.vector.tensor_scalar_mul(
                        vt[:, h * 65 : h * 65 + 64], vps[:, h * 64 : (h + 1) * 64],
                        rms_tok[:, b * NKT + kt : b * NKT + kt + 1],
                    )
                    nc.gpsimd.memset(vt[:, h * 65 + 64 : h * 65 + 65], 1.0)
                v_t[(b, kt)] = vt
                if kt % 2 == 1:
                    yield

        # ---- attention ----
        attn_sb = [singles.tile([128, T], BF, tag=f"attn{b}", name=f"attn{b}")
                   for b in range(B)]
        wo_sb = singles.tile([128, 8 * D], BF)
        xres_sb = singles.tile([128, 4 * D], BF)
        a2a_in = [dram.tile([1024, TD], BF, tag=f"a2ain{b}", name=f"a2ain{b}")
                  for b in range(B)]
        a2a_out = [dram.tile([1024, TD], BF, tag=f"a2aout{b}", name=f"a2aout{b}")
                   for b in range(B)]

        fillers = []          # queue of 0-arg closures emitting ~0.5us of work

        def run_filler(n=1):
            for _ in range(n):
                if fillers:
                    fillers.pop(0)()

        def emit_st(b, qg, kt):
            q0 = b * T + qg * QG
            st = st_ps.tile([128, 2 * QG], F32, tag="st", name="st")
            for h in range(HPC):
                o = h * 64
                nc.tensor.matmul(
                    st[:, h * QG : (h + 1) * QG],
                    lhsT=blocks[1][o : o + 64,
                                   b * T + kt * 128 : b * T + (kt + 1) * 128],
                    rhs=blocks[0][o : o + 64, q0 : q0 + QG],
                    start=True, stop=True,
                )
            ex = exp_pool.tile([128, 2 * QG], BF, tag="ex", name="ex")
            nc.scalar.activation(
                ex[:], st[:], mybir.ActivationFunctionType.Exp,
                scale=rms_tok[:, b * NKT + kt : b * NKT + kt + 1],
            )
            return ex

        def attn_qg(b, qg):
            av = av_ps.tile([128, 2 * QG], F32, tag="av", name="av")
            ex_cur = emit_st(b, qg, 0)
            for kt in range(NKT):
                ex_next = emit_st(b, qg, kt + 1) if kt + 1 < NKT else None
                run_filler()
                vt = v_t[(b, kt)]
                for h in range(HPC):
                    for qs in range(4):
                        c0 = h * QG + qs * 65
                        nc.tensor.matmul(
                            av[:, c0 : c0 + 65],
                            lhsT=ex_cur[:, h * QG + qs * 128
                                        : h * QG + (qs + 1) * 128],
                            rhs=vt[:, h * 65 : (h + 1) * 65],
                            start=(kt == 0 and qs == 0),
                            stop=(kt == NKT - 1 and qs == 3),
                        )
                ex_cur = ex_next
            # epilogue: reciprocal of denominators, normalize, transpose back
            rc = epi_pool.tile([128, 8], F32, tag="rc", name="rc")
            dsb = epi_pool.tile([128, 8], F32, tag="dsb", name="dsb")
            den_ap = bass.AP(tensor=av.tensor, offset=av.offset + 64,
                             ap=[list(av.ap[0]), [QG, 2], [65, 4]])
            nc.vector.tensor_copy(dsb.rearrange("p (h j) -> p h j", h=2), den_ap)
            nc.vector.reciprocal(rc[:], dsb[:])
            for qs in range(4):
                at = epi_pool.tile([128, 128], BF, tag=f"at{qs}", name=f"at{qs}")
                for h in range(HPC):
                    c0 = h * QG + qs * 65
                    nc.vector.tensor_scalar_mul(
                        at[:, h * 64 : (h + 1) * 64], av[:, c0 : c0 + 64],
                        rc[:, h * 4 + qs : h * 4 + qs + 1],
                    )
                abt = mm_ps.tile([128, 128], BF, tag="mmps", name="abt")
                nc.tensor.transpose(abt[:], at[:], ident[:])
                nc.vector.tensor_copy(
                    attn_sb[b][:, qg * QG + qs * 128 : qg * QG + (qs + 1) * 128],
                    abt[:],
                )
            # stage the two finished a2a chunks for this qg
            for j in (2 * qg, 2 * qg + 1):
                nc.sync.dma_start(
                    a2a_in[b][j * 128 : (j + 1) * 128, :],
                    attn_sb[b][:, j * TD : (j + 1) * TD],
                )

        def do_a2a(b):
            if with_collective:
                nc.gpsimd.collective_compute(
                    "AllToAll", mybir.AluOpType.bypass,
                    replica_groups=[list(range(N_CORES))],
                    ins=[a2a_in[b].opt()], outs=[a2a_out[b].opt()],
                )
            else:
                nc.sync.dma_start(a2a_out[b][:], a2a_in[b][:])

        attn_all = [None, None]

        def load_attn_all(b):
            t = singles.tile([128, 8 * TD], BF, tag=f"aall{b}", name=f"aall{b}")
            nc.sync.dma_start(
                t.rearrange("p (c x) -> p c x", c=8),
                a2a_out[b].rearrange("(c p) x -> p c x", c=8),
            )
            attn_all[b] = t

        def out_proj_units(b):
            for ttl in range(TD // 128):
                ot = out_sb.tile([128, D], BF, tag="ot", name="ot")
                for jh in range(2):
                    ps = mm_ps.tile([128, 512], F32, tag="mmps", name="ops")
                    for ch in range(8):
                        nc.tensor.matmul(
                            ps[:],
                            lhsT=attn_all[b][:, ch * TD + ttl * 128
                                             : ch * TD + (ttl + 1) * 128],
                            rhs=wo_sb[:, ch * D + jh * 512
                                      : ch * D + (jh + 1) * 512],
                            start=(ch == 0), stop=(ch == 7),
                        )
                    nc.vector.tensor_add(
                        ot[:, jh * 512 : (jh + 1) * 512], ps[:],
                        xres_sb[:, (b * 2 + ttl) * D + jh * 512
                                : (b * 2 + ttl) * D + (jh + 1) * 512],
                    )
                    yield
                nc.sync.dma_start(
                    io["y"][(b * 2 + ttl) * 128 : (b * 2 + ttl + 1) * 128, :],
                    ot[:],
                )

        # ---- schedule ----
        # Phase 1: sumsq (X^T.X) for all of batch 0 first, so the rms
        # reciprocal + broadcast round trip overlaps the qkv projections.
        xsl_tiles = {0: xsl_load(0, staggered=True)}
        load_consts()
        xsl_tiles[1] = xsl_load(1)
        xsl_tiles[2] = xsl_load(2)
        for tg in range(4):
            if tg + 1 < 4 and tg + 1 not in xsl_tiles:
                xsl_tiles[tg + 1] = xsl_load(tg + 1)
            for _ in tg_xtx_units(tg, xsl_tiles[tg]):
                pass
        build_rms(0)
        for tg in range(4):
            for _ in tg_qkv_units(tg, xsl_tiles.pop(tg)):
                pass
        xsl_tiles[4] = xsl_load(4)
        for _ in build_vt_units(0):
            pass
        for tg in range(4):
            q_rms(tg)

        # fillers for batch-0 attention: batch-1 prep (sumsq first so the
        # batch-1 rms broadcast round trip is issued as early as possible)
        def gen_fillers_b0():
            xsl_tiles[5] = xsl_load(5)
            for tg in range(4, 8):
                if tg + 2 < 8:
                    xsl_tiles[tg + 2] = xsl_load(tg + 2)
                yield from tg_xtx_units(tg, xsl_tiles[tg])
            build_rms(1)
            yield
            for tg in range(4, 8):
                yield from tg_qkv_units(tg, xsl_tiles.pop(tg))
            for tg in range(4, 8):
                q_rms(tg)
            yield
            yield from build_vt_units(1)
            nc.sync.dma_start(
                wo_sb.rearrange("p (c j) -> p c j", c=8),
                io["wo"].rearrange("c p j -> p c j"),
            )
            nc.sync.dma_start(
                xres_sb.rearrange("p (a d) -> p a d", a=4),
                io["xres"].rearrange("(a p) d -> p a d", p=128),
            )
            yield

        def push(gen):
            def f():
                try:
                    next(gen)
                except StopIteration:
                    pass
                else:
                    fillers.append(f)
            fillers.append(f)

        push(gen_fillers_b0())

        for qg in range(NQG):
            attn_qg(0, qg)
        # drain any remaining batch-1 prep
        while fillers:
            run_filler()
        do_a2a(0)
        load_attn_all(0)

        # batch-1 attention; out-projection 0 runs after, covering the
        # a2a(1) + gather latency so the PE never idles (pstate stays high)
        for qg in range(NQG):
            attn_qg(1, qg)
        do_a2a(1)
        load_attn_all(1)
        for _ in out_proj_units(0):
            pass
        # keep the PE ramped while the batch-1 gather lands
        for w in range(20):
            wj = mm_ps.tile([128, 512], F32, tag="mmps", name="warm")
            nc.tensor.matmul(wj[:], lhsT=ident[:], rhs=blocks[0][:, 0:512],
                             start=True, stop=True)
        for _ in out_proj_units(1):
            pass


def _prep_inputs(x, norm_w, w_qkv, w_out):
    """Host-side sharding. Returns list of per-core input dicts."""
    bf16 = ml_dtypes.bfloat16
    xf = np.ascontiguousarray(x.reshape(BT, D).astype(np.float32))
    xT = np.ascontiguousarray(xf.T).reshape(8, 128, BT).astype(bf16)

    w_eff = w_qkv.astype(np.float32) * norm_w.astype(np.float32)[None, :]
    scale = HD ** -0.5
    # rope tables (plain sin; rotation signs live in rotT)
    inv = 1.0 / (ROPE_BASE ** (np.arange(0, HD, 2, dtype=np.float32) / HD))
    t = np.arange(T, dtype=np.float32)
    fr = t[:, None] * inv[None, :]
    emb = np.concatenate([fr, fr], axis=-1)          # [T, 64]
    cosT = np.cos(emb).T                             # [64, T]
    sinT = np.sin(emb).T
    cos_b = np.concatenate([cosT, cosT], axis=0)     # [128, T]
    sin_b = np.concatenate([sinT, sinT], axis=0)

    # rotate-half as a matmul: rot(x)[i] = -x[i+32] (i<32), x[i-32] (i>=32)
    R64 = np.zeros((64, 64), dtype=np.float32)
    R64[np.arange(32), np.arange(32) + 32] = -1.0
    R64[np.arange(32, 64), np.arange(32)] = 1.0
    Rfull = np.zeros((128, 128), dtype=np.float32)
    Rfull[0:64, 0:64] = R64
    Rfull[64:128, 64:128] = R64
    rotT = np.ascontiguousarray(Rfull.T)
    tables = np.concatenate([cos_b, sin_b, rotT], axis=1).astype(bf16)

    woT = np.ascontiguousarray(w_out.astype(np.float32).T)      # [1024 k, 1024 j]
    wo = woT.reshape(8, 128, D).astype(bf16)

    in_maps = []
    for c in range(N_CORES):
        h0, h1 = 2 * c, 2 * c + 1
        rows = []
        for part, s in ((0, scale), (1, 1.0), (2, 1.0)):  # q, k, v
            for h in (h0, h1):
                r = w_eff[part * D + h * HD : part * D + (h + 1) * HD] * s
                rows.append(r)
        wc = np.concatenate(rows, axis=0)            # [384, 1024]
        wqc = np.ascontiguousarray(wc.T).reshape(8, 128, JC).astype(bf16)
        xres = np.concatenate(
            [xf[c * TD : (c + 1) * TD], xf[T + c * TD : T + (c + 1) * TD]],
            axis=0,
        ).astype(bf16)
        in_maps.append({
            "xT": xT, "wq": wqc, "wo": wo, "tables": tables, "xres": xres,
        })
    return in_maps


_CACHE = {}


def _get_compiled():
    if "nc" not in _CACHE:
        nc = bacc.Bacc("TRN2", target_bir_lowering=False, debug=False,
                       num_devices=N_CORES)
        build_kernel(nc)
        nc.compile()
        _CACHE["nc"] = nc
    return _CACHE["nc"]


def kernel(x, norm_w, w_qkv, w_out):
    nc = _get_compiled()
    in_maps = _prep_inputs(np.asarray(x), np.asarray(norm_w),
                           np.asarray(w_qkv), np.asarray(w_out))
    res = run_bass_kernel_spmd(nc, in_maps, list(range(N_CORES)))
    y = np.empty((BT, D), dtype=np.float32)
    for c in range(N_CORES):
        yc = np.asarray(res.results[c]["y"], dtype=np.float32)
        y[c * TD : (c + 1) * TD] = yc[0:TD]
        y[2048 + c * TD : 2048 + (c + 1) * TD] = yc[TD : 2 * TD]
    return y.reshape(B, T, D).astype(np.float32)


# revision 38
# speedup vs baseline: 1.0324x; 1.0324x over previous
"""Bidirectional attention block (RMSNorm -> QKV+RoPE -> SDPA -> out-proj -> residual)
on 8 Trainium2 NeuronCores.

Sharding: tensor-parallel over heads (2 heads/core) through attention, then two
per-batch on-device AllToAlls switch to token-parallel (2x256 tokens/core) for
the output projection + residual. Host only slices/concatenates numpy arrays.

Key structure (v3):
- x loaded ONCE (feature-major xT), one batched DMA per 512-token group.
- Token sum-of-squares for RMSNorm computed on the PE via X^T.X diagonal
  (DVE multiply-by-identity + row reduce extracts the diag).
- RoPE rotate-half via a PE matmul with a constant +-1 permutation matrix.
- v transposed to token-major via PE transposes (not DMA).
- Attention AV uses the transposed orientation: out[q, hd] accumulates in PSUM
  with an extra ones-column producing the softmax denominator; normalization is
  a per-partition broadcast multiply (no DMA round trips).
- The kt loop is software-pipelined (scores for kt+1 issue before AV of kt) and
  batch-1 prep / batch-0 out-projection are sliced into small "filler" units
  interleaved between attention iterations to keep the PE busy during exp.

Shapes hardcoded for B=2, T=2048, D_MODEL=1024, N_HEADS=16, HEAD_DIM=64.
"""

import numpy as np
import ml_dtypes

import concourse.bass as bass
import concourse.tile as tile
from concourse import bacc, mybir
from concourse.bass_utils import run_bass_kernel_spmd
from concourse.masks import make_identity

B, T, D = 2, 2048, 1024
H, HD = 16, 64
BT = B * T                      # 4096 tokens
N_CORES = 8
HPC = H // N_CORES              # 2 heads per core
JC = 3 * HPC * HD               # 384 qkv features per core
TD = BT // N_CORES // B         # 256 tokens per core per batch (stage D)
RMS_EPS = 1e-5
ROPE_BASE = 10000.0

BF = mybir.dt.bfloat16
F32 = mybir.dt.float32

QG = 512                        # queries per attention group
NQG = T // QG                   # 4 q-groups per batch
NKT = T // 128                  # 16 k-tiles per batch
NTT = BT // 128                 # 32 token tiles globally


def build_kernel(nc, with_collective=True):
    xT_ap = nc.dram_tensor("xT", [8, 128, BT], BF, kind="ExternalInput").ap()
    wq_ap = nc.dram_tensor("wq", [8, 128, JC], BF, kind="ExternalInput").ap()
    wo_ap = nc.dram_tensor("wo", [8, 128, D], BF, kind="ExternalInput").ap()
    tab_ap = nc.dram_tensor("tables", [128, 2 * T + 128], BF,
                            kind="ExternalInput").ap()
    xres_ap = nc.dram_tensor("xres", [2 * TD, D], BF, kind="ExternalInput").ap()
    y_ap = nc.dram_tensor("y", [2 * TD, D], BF, kind="ExternalOutput").ap()

    rrow_d = nc.dram_tensor("rms_row", [BT], BF).ap()

    with tile.TileContext(nc) as tc:
        _body(nc, tc, dict(
            xT=xT_ap, wq=wq_ap, wo=wo_ap, tab=tab_ap,
            xres=xres_ap, y=y_ap, rrow_d=rrow_d,
        ), with_collective)
    return nc


def _body(nc, tc, io, with_collective):
    from contextlib import ExitStack
    ctx = ExitStack()
    with ctx:
        singles = ctx.enter_context(tc.tile_pool(name="singles", bufs=1))
        xsl_pool = ctx.enter_context(tc.tile_pool(name="xsl", bufs=4))
        rope_tmp = ctx.enter_context(tc.tile_pool(name="rope_tmp", bufs=2))
        exp_pool = ctx.enter_context(tc.tile_pool(name="exp", bufs=4))
        epi_pool = ctx.enter_context(tc.tile_pool(name="epi", bufs=2))
        out_sb = ctx.enter_context(tc.tile_pool(name="out_sb", bufs=2))
        dram = ctx.enter_context(tc.tile_pool(name="dram", bufs=1, space="DRAM"))
        mm_ps = ctx.enter_context(tc.tile_pool(name="mm_ps", bufs=2, space="PSUM"))
        st_ps = ctx.enter_context(tc.tile_pool(name="st_ps", bufs=2, space="PSUM"))
        av_ps = ctx.enter_context(tc.tile_pool(name="av_ps", bufs=1, space="PSUM"))

        # ---- constants (batched loads; xsl0 issued first in the schedule) ----
        ident = singles.tile([128, 128], BF)
        make_identity(nc, ident)
        ident32 = singles.tile([128, 128], F32)
        make_identity(nc, ident32)
        tab_sb = singles.tile([128, 2 * T + 128], BF)
        wq_sb = singles.tile([128, 8 * JC], BF)

        def load_consts():
            nc.sync.dma_start(
                wq_sb.rearrange("p (c j) -> p c j", c=8),
                io["wq"].rearrange("c p j -> p c j"),
            )
            nc.sync.dma_start(tab_sb[:], io["tab"][:])

        cos_sb = tab_sb[:, 0:T]
        sin_sb = tab_sb[:, T : 2 * T]
        rotT = tab_sb[:, 2 * T : 2 * T + 128]
        eps_t = singles.tile([128, 1], F32)
        nc.vector.memset(eps_t[:], RMS_EPS)
        sumsq = singles.tile([128, NTT], F32)
        rms_tok = singles.tile([128, NTT], F32)
        rms_bc = [None, None]

        # ---- stage B: qkv projection + RoPE + sumsq via X^T.X diag ----
        blocks = []  # q, k, v blocks [128, BT] bf16
        for jt in range(3):
            blocks.append(singles.tile([128, BT], BF, tag=f"blk{jt}", name=f"blk{jt}"))

        def xsl_load(tg, staggered=False):
            g = slice(tg * 512, (tg + 1) * 512)
            t = xsl_pool.tile([128, 8 * 512], BF, tag="xsl", name="xsl")
            if staggered:
                for ch in range(8):
                    nc.sync.dma_start(t[:, ch * 512 : (ch + 1) * 512],
                                      io["xT"][ch, :, g])
            else:
                nc.sync.dma_start(
                    t.rearrange("p (c x) -> p c x", c=8),
                    io["xT"][:, :, g].rearrange("c p x -> p c x"),
                )
            return t

        def tg_xtx_units(tg, xsl):
            """sumsq for this tg's 4 token tiles: diag of X^T X."""
            for s in range(4):
                gt = tg * 4 + s
                ss = slice(s * 128, (s + 1) * 128)
                xtx = mm_ps.tile([128, 128], F32, tag="mmps", name="xtx")
                for ch in range(8):
                    nc.tensor.matmul(
                        xtx[:], lhsT=xsl[:, ch * 512 : (ch + 1) * 512][:, ss],
                        rhs=xsl[:, ch * 512 : (ch + 1) * 512][:, ss],
                        start=(ch == 0), stop=(ch == 7),
                    )
                junk = epi_pool.tile([128, 128], F32, tag="junk", name="junk")
                nc.vector.tensor_mul(junk[:], xtx[:], ident32[:])
                nc.vector.tensor_reduce(
                    sumsq[:, gt : gt + 1], junk[:],
                    axis=mybir.AxisListType.X, op=mybir.AluOpType.add,
                )
                yield

        def tg_qkv_units(tg, xsl):
            """qkv projection then RoPE for one 512-token group."""
            b = tg // (T // 512)
            tl = tg * 512 - b * T          # batch-local col offset
            g = slice(tg * 512, (tg + 1) * 512)
            cs = slice(tl, tl + 512)
            for jt in range(3):
                ps = mm_ps.tile([128, 512], F32, tag="mmps", name="qkvps")
                for ch in range(8):
                    nc.tensor.matmul(
                        ps[:],
                        lhsT=wq_sb[:, ch * JC + jt * 128 : ch * JC + (jt + 1) * 128],
                        rhs=xsl[:, ch * 512 : (ch + 1) * 512],
                        start=(ch == 0), stop=(ch == 7),
                    )
                nc.vector.tensor_copy(blocks[jt][:, g], ps[:])
                yield
            # RoPE on q, k slices in place
            for jt in (0, 1):
                blk = blocks[jt]
                rot_ps = mm_ps.tile([128, 512], F32, tag="mmps", name="rotps")
                nc.tensor.matmul(rot_ps[:], lhsT=rotT, rhs=blk[:, g],
                                 start=True, stop=True)
                m1 = rope_tmp.tile([128, 512], BF, tag="m1", name="m1")
                nc.gpsimd.tensor_mul(m1[:], blk[:, g], cos_sb[:, cs])
                m2 = rope_tmp.tile([128, 512], BF, tag="m2", name="m2")
                nc.vector.tensor_mul(m2[:], rot_ps[:], sin_sb[:, cs])
                nc.vector.tensor_add(blk[:, g], m1[:], m2[:])
                yield

        def build_rms(b):
            bs = slice(b * (NTT // 2), (b + 1) * (NTT // 2))
            sqv = epi_pool.tile([128, NTT // 2], F32, tag="sqv", name="sqv")
            nc.scalar.activation(
                sqv[:], sumsq[:, bs], mybir.ActivationFunctionType.Sqrt,
                bias=eps_t[:], scale=1.0 / D,
            )
            nc.vector.reciprocal(rms_tok[:, bs], sqv[:])
            rtr_ps = mm_ps.tile([NTT // 2, 128], F32, tag="mmps", name="rtr")
            nc.tensor.transpose(rtr_ps[:], rms_tok[:, bs], ident32[:])
            rms_tr = epi_pool.tile([NTT // 2, 128], BF, tag="rmstr", name="rmstr")
            nc.vector.tensor_copy(rms_tr[:], rtr_ps[:])
            nc.sync.dma_start(
                io["rrow_d"].rearrange("(x k p) -> x k p", x=B, p=128)[b],
                rms_tr[:],
            )
            rbc = singles.tile([128, T], BF, tag=f"rmsbc{b}", name=f"rmsbc{b}")
            rrow_bcast = bass.AP(
                tensor=io["rrow_d"].tensor, offset=io["rrow_d"].offset + b * T,
                ap=[[0, 128], [1, T]],
            )
            nc.gpsimd.dma_start(rbc[:], rrow_bcast)
            rms_bc[b] = rbc

        def q_rms(tg):
            b = tg // (T // 512)
            tl = tg * 512 - b * T
            g = slice(tg * 512, (tg + 1) * 512)
            nc.vector.tensor_mul(blocks[0][:, g], blocks[0][:, g],
                                 rms_bc[b][:, tl : tl + 512])

        # ---- v transpose to token-major [128 tok, 2*(64 v + ones)] ----
        v_t = {}

        def build_vt_units(b):
            for kt in range(NKT):
                vps = mm_ps.tile([128, 128], BF, tag="mmps", name="vps")
                nc.tensor.transpose(
                    vps[:],
                    blocks[2][:, b * T + kt * 128 : b * T + (kt + 1) * 128],
                    ident[:],
                )
                vt = singles.tile([128, 130], BF, tag=f"vt{b}_{kt}",
                                  name=f"vt{b}_{kt}")
                for h in range(2):
                    nc.vector.tensor_scalar_mul(
                        vt[:, h * 65 : h * 65 + 64], vps[:, h * 64 : (h + 1) * 64],
                        rms_tok[:, b * NKT + kt : b * NKT + kt + 1],
                    )
                    nc.gpsimd.memset(vt[:, h * 65 + 64 : h * 65 + 65], 1.0)
                v_t[(b, kt)] = vt
                if kt % 2 == 1:
                    yield

        # ---- attention ----
        attn_sb = [singles.tile([128, T], BF, tag=f"attn{b}", name=f"attn{b}")
                   for b in range(B)]
        wo_sb = singles.tile([128, 8 * D], BF)
        xres_sb = singles.tile([128, 4 * D], BF)
        a2a_in = [dram.tile([1024, TD], BF, tag=f"a2ain{b}", name=f"a2ain{b}")
                  for b in range(B)]
        a2a_out = [dram.tile([1024, TD], BF, tag=f"a2aout{b}", name=f"a2aout{b}")
                   for b in range(B)]

        fillers = []          # queue of 0-arg closures emitting ~0.5us of work

        def run_filler(n=1):
            for _ in range(n):
                if fillers:
                    fillers.pop(0)()

        def emit_st(b, qg, kt):
            q0 = b * T + qg * QG
            st = st_ps.tile([128, 2 * QG], F32, tag="st", name="st")
            for h in range(HPC):
                o = h * 64
                nc.tensor.matmul(
                    st[:, h * QG : (h + 1) * QG],
                    lhsT=blocks[1][o : o + 64,
                                   b * T + kt * 128 : b * T + (kt + 1) * 128],
                    rhs=blocks[0][o : o + 64, q0 : q0 + QG],
                    start=True, stop=True,
                )
            ex = exp_pool.tile([128, 2 * QG], BF, tag="ex", name="ex")
            nc.scalar.activation(
                ex[:], st[:], mybir.ActivationFunctionType.Exp,
                scale=rms_tok[:, b * NKT + kt : b * NKT + kt + 1],
            )
            return ex

        def attn_qg(b, qg):
            av = av_ps.tile([128, 2 * QG], F32, tag="av", name="av")
            ex_cur = emit_st(b, qg, 0)
            for kt in range(NKT):
                ex_next = emit_st(b, qg, kt + 1) if kt + 1 < NKT else None
                run_filler()
                vt = v_t[(b, kt)]
                for h in range(HPC):
                    for qs in range(4):
                        c0 = h * QG + qs * 65
                        nc.tensor.matmul(
                            av[:, c0 : c0 + 65],
                            lhsT=ex_cur[:, h * QG + qs * 128
                                        : h * QG + (qs + 1) * 128],
                            rhs=vt[:, h * 65 : (h + 1) * 65],
                            start=(kt == 0 and qs == 0),
                            stop=(kt == NKT - 1 and qs == 3),
                        )
                ex_cur = ex_next
            # epilogue: reciprocal of denominators, normalize, transpose back
            rc = epi_pool.tile([128, 8], F32, tag="rc", name="rc")
            dsb = epi_pool.tile([128, 8], F32, tag="dsb", name="dsb")
            den_ap = bass.AP(tensor=av.tensor, offset=av.offset + 64,
                             ap=[list(av.ap[0]), [QG, 2], [65, 4]])
            nc.vector.tensor_copy(dsb.rearrange("p (h j) -> p h j", h=2), den_ap)
            nc.vector.reciprocal(rc[:], dsb[:])
            for qs in range(4):
                at = epi_pool.tile([128, 128], BF, tag=f"at{qs}", name=f"at{qs}")
                for h in range(HPC):
                    c0 = h * QG + qs * 65
                    nc.vector.tensor_scalar_mul(
                        at[:, h * 64 : (h + 1) * 64], av[:, c0 : c0 + 64],
                        rc[:, h * 4 + qs : h * 4 + qs + 1],
                    )
                abt = mm_ps.tile([128, 128], BF, tag="mmps", name="abt")
                nc.tensor.transpose(abt[:], at[:], ident[:])
                nc.vector.tensor_copy(
                    attn_sb[b][:, qg * QG + qs * 128 : qg * QG + (qs + 1) * 128],
                    abt[:],
                )
            # stage the two finished a2a chunks for this qg
            for j in (2 * qg, 2 * qg + 1):
                nc.sync.dma_start(
                    a2a_in[b][j * 128 : (j + 1) * 128, :],
                    attn_sb[b][:, j * TD : (j + 1) * TD],
                )

        def do_a2a(b):
            if with_collective:
                nc.gpsimd.collective_compute(
                    "AllToAll", mybir.AluOpType.bypass,
                    replica_groups=[list(range(N_CORES))],
                    ins=[a2a_in[b].opt()], outs=[a2a_out[b].opt()],
                )
            else:
                nc.sync.dma_start(a2a_out[b][:], a2a_in[b][:])

        attn_all = [None, None]

        def load_attn_all(b):
            t = singles.tile([128, 8 * TD], BF, tag=f"aall{b}", name=f"aall{b}")
            nc.sync.dma_start(
                t.rearrange("p (c x) -> p c x", c=8),
                a2a_out[b].rearrange("(c p) x -> p c x", c=8),
            )
            attn_all[b] = t

        def out_proj_units(b):
            for ttl in range(TD // 128):
                ot = out_sb.tile([128, D], BF, tag="ot", name="ot")
                for jh in range(2):
                    ps = mm_ps.tile([128, 512], F32, tag="mmps", name="ops")
                    for ch in range(8):
                        nc.tensor.matmul(
                            ps[:],
                            lhsT=attn_all[b][:, ch * TD + ttl * 128
                                             : ch * TD + (ttl + 1) * 128],
                            rhs=wo_sb[:, ch * D + jh * 512
                                      : ch * D + (jh + 1) * 512],
                            start=(ch == 0), stop=(ch == 7),
                        )
                    nc.vector.tensor_add(
                        ot[:, jh * 512 : (jh + 1) * 512], ps[:],
                        xres_sb[:, (b * 2 + ttl) * D + jh * 512
                                : (b * 2 + ttl) * D + (jh + 1) * 512],
                    )
                    yield
                nc.sync.dma_start(
                    io["y"][(b * 2 + ttl) * 128 : (b * 2 + ttl + 1) * 128, :],
                    ot[:],
                )

        # ---- schedule ----
        # Phase 1: sumsq (X^T.X) for all of batch 0 first, so the rms
        # reciprocal + broadcast round trip overlaps the qkv projections.
        xsl_tiles = {0: xsl_load(0, staggered=True)}
        load_consts()
        xsl_tiles[1] = xsl_load(1)
        xsl_tiles[2] = xsl_load(2)
        for tg in range(2):
            for _ in tg_xtx_units(tg, xsl_tiles[tg]):
                pass
            if tg == 0:
                xsl_tiles[3] = xsl_load(3)
            for _ in tg_qkv_units(tg, xsl_tiles[tg]):
                pass
        for tg in (2, 3):
            for _ in tg_xtx_units(tg, xsl_tiles[tg]):
                pass
        build_rms(0)
        for tg in (2, 3):
            for _ in tg_qkv_units(tg, xsl_tiles[tg]):
                pass
        for tg in range(4):
            xsl_tiles.pop(tg)
        xsl_tiles[4] = xsl_load(4)
        for _ in build_vt_units(0):
            pass
        for tg in range(4):
            q_rms(tg)

        # fillers for batch-0 attention: batch-1 prep (sumsq first so the
        # batch-1 rms broadcast round trip is issued as early as possible)
        def gen_fillers_b0():
            xsl_tiles[5] = xsl_load(5)
            for tg in range(4, 8):
                if tg + 2 < 8:
                    xsl_tiles[tg + 2] = xsl_load(tg + 2)
                yield from tg_xtx_units(tg, xsl_tiles[tg])
            build_rms(1)
            yield
            for tg in range(4, 8):
                yield from tg_qkv_units(tg, xsl_tiles.pop(tg))
            for tg in range(4, 8):
                q_rms(tg)
            yield
            yield from build_vt_units(1)
            nc.sync.dma_start(
                wo_sb.rearrange("p (c j) -> p c j", c=8),
                io["wo"].rearrange("c p j -> p c j"),
            )
            nc.sync.dma_start(
                xres_sb.rearrange("p (a d) -> p a d", a=4),
                io["xres"].rearrange("(a p) d -> p a d", p=128),
            )
            yield

        def push(gen):
            def f():
                try:
                    next(gen)
                except StopIteration:
                    pass
                else:
                    fillers.append(f)
            fillers.append(f)

        push(gen_fillers_b0())

        for qg in range(NQG):
            attn_qg(0, qg)
        # drain any remaining batch-1 prep
        while fillers:
            run_filler()
        do_a2a(0)
        load_attn_all(0)

        # batch-1 attention; out-projection 0 runs after, covering the
        # a2a(1) + gather latency so the PE never idles (pstate stays high)
        for qg in range(NQG):
            attn_qg(1, qg)
        do_a2a(1)
        load_attn_all(1)
        for _ in out_proj_units(0):
            pass
        # keep the PE ramped while the batch-1 gather lands
        for w in range(20):
            wj = mm_ps.tile([128, 512], F32, tag="mmps", name="warm")
            nc.tensor.matmul(wj[:], lhsT=ident[:], rhs=blocks[0][:, 0:512],
                             start=True, stop=True)
        for _ in out_proj_units(1):
            pass


def _prep_inputs(x, norm_w, w_qkv, w_out):
    """Host-side sharding. Returns list of per-core input dicts."""
    bf16 = ml_dtypes.bfloat16
    xf = np.ascontiguousarray(x.reshape(BT, D).astype(np.float32))
    xT = np.ascontiguousarray(xf.T).reshape(8, 128, BT).astype(bf16)

    w_eff = w_qkv.astype(np.float32) * norm_w.astype(np.float32)[None, :]
    scale = HD ** -0.5
    # rope tables (plain sin; rotation signs live in rotT)
    inv = 1.0 / (ROPE_BASE ** (np.arange(0, HD, 2, dtype=np.float32) / HD))
    t = np.arange(T, dtype=np.float32)
    fr = t[:, None] * inv[None, :]
    emb = np.concatenate([fr, fr], axis=-1)          # [T, 64]
    cosT = np.cos(emb).T                             # [64, T]
    sinT = np.sin(emb).T
    cos_b = np.concatenate([cosT, cosT], axis=0)     # [128, T]
    sin_b = np.concatenate([sinT, sinT], axis=0)

    # rotate-half as a matmul: rot(x)[i] = -x[i+32] (i<32), x[i-32] (i>=32)
    R64 = np.zeros((64, 64), dtype=np.float32)
    R64[np.arange(32), np.arange(32) + 32] = -1.0
    R64[np.arange(32, 64), np.arange(32)] = 1.0
    Rfull = np.zeros((128, 128), dtype=np.float32)
    Rfull[0:64, 0:64] = R64
    Rfull[64:128, 64:128] = R64
    rotT = np.ascontiguousarray(Rfull.T)
    tables = np.concatenate([cos_b, sin_b, rotT], axis=1).astype(bf16)

    woT = np.ascontiguousarray(w_out.astype(np.float32).T)      # [1024 k, 1024 j]
    wo = woT.reshape(8, 128, D).astype(bf16)

    in_maps = []
    for c in range(N_CORES):
        h0, h1 = 2 * c, 2 * c + 1
        rows = []
        for part, s in ((0, scale), (1, 1.0), (2, 1.0)):  # q, k, v
            for h in (h0, h1):
                r = w_eff[part * D + h * HD : part * D + (h + 1) * HD] * s
                rows.append(r)
        wc = np.concatenate(rows, axis=0)            # [384, 1024]
        wqc = np.ascontiguousarray(wc.T).reshape(8, 128, JC).astype(bf16)
        xres = np.concatenate(
            [xf[c * TD : (c + 1) * TD], xf[T + c * TD : T + (c + 1) * TD]],
            axis=0,
        ).astype(bf16)
        in_maps.append({
            "xT": xT, "wq": wqc, "wo": wo, "tables": tables, "xres": xres,
        })
    return in_maps


_CACHE = {}


def _get_compiled():
    if "nc" not in _CACHE:
        nc = bacc.Bacc("TRN2", target_bir_lowering=False, debug=False,
                       num_devices=N_CORES)
        build_kernel(nc)
        nc.compile()
        _CACHE["nc"] = nc
    return _CACHE["nc"]


def kernel(x, norm_w, w_qkv, w_out):
    nc = _get_compiled()
    in_maps = _prep_inputs(np.asarray(x), np.asarray(norm_w),
                           np.asarray(w_qkv), np.asarray(w_out))
    res = run_bass_kernel_spmd(nc, in_maps, list(range(N_CORES)))
    y = np.empty((BT, D), dtype=np.float32)
    for c in range(N_CORES):
        yc = np.asarray(res.results[c]["y"], dtype=np.float32)
        y[c * TD : (c + 1) * TD] = yc[0:TD]
        y[2048 + c * TD : 2048 + (c + 1) * TD] = yc[TD : 2 * TD]
    return y.reshape(B, T, D).astype(np.float32)
